# revision 45
# baseline (speedup 1.0000x reference)
"""Trainium2 Bass kernel for nn_Attention_Critic (gnn_message_passing).

Strategy (8-way batch data parallel, 4096 samples/core):
  - Host fuses weights: WeQ=We@Wq, WeK=We@Wk, WeV=We@Wv (encoder folded into
    the Q/K/V projections; enc is materialized only for agent 0),
    WoW1b=Wo@W1[256:], b1f=b1+bo@W1[256:] (fc_out folded into l1).
  - Final LN+l3 folded algebraically: res = rstd*(W3.qr - mean*sum(W3)) + b3,
    with mean/var of qr from ones/W3 matvecs (float32r).
  - Per 512-sample chunk: LN via bn_stats + tensor_scalar (sample-major),
    PE-transpose to feature-major [feat, sample] bf16, projections as
    128x128-tiled bf16 matmuls, attention scores via elementwise QK + a
    block-ones matmul that reduces over head dims AND broadcasts the score
    back across them, exp on ScalarE.
  - Softmax denominator computed compactly: per-head ones-matvecs reduce the
    broadcast exp over head dims AND accumulate over the 7 keys in PSUM;
    reciprocal_approx_fast on the compact [4, NB] form; PE indicator-matmul
    broadcasts 1/Z back to all 256 dims (no SDMA tree, no wide reciprocal).
  - avU k-reduction via SDMA CCE accumulate (frees DVE); leaky-relu as one
    scalar_tensor_tensor (mult+max) op.
"""

import contextlib

import numpy as np
import ml_dtypes

import concourse.bass as bass
import concourse.tile as tile
from concourse import bacc, mybir
from concourse.bass_utils import run_bass_kernel_spmd
from concourse.masks import make_identity

AF = mybir.ActivationFunctionType
OP = mybir.AluOpType
BF = mybir.dt.bfloat16
F32 = mybir.dt.float32

B, A, S, D, H, NH, HD = 32768, 8, 256, 256, 256, 4, 64
EPS = 1e-5
NCORES = 8
BC = B // NCORES          # 4096 samples per core
NB = 512                  # samples per chunk
NCH = BC // NB            # 8 chunks per core
NW = 1538 + 2 + 256       # fused bf16 weight columns (+Wz, +Wb)
NBI = 7                   # f32 bias/vec slots

# wcat column offsets (each 256 wide)
C_WEW1A, C_WQ, C_WK, C_WV, C_WO1B, C_L, C_W3O = (
    0, 256, 512, 768, 1024, 1280, 1536)
C_Z = 1538                # [128, 2] per-head ones/64 reducer (ks=0 plane)
C_B = 1540                # [64, 2*128] head->dim broadcast indicator
# bcat slots: biases 0-4, W3 at 5, ones at 6
B_BE, B_BQ, B_BK, B_BV, B_B1F, B_W3, B_ONE = 0, 1, 2, 3, 4, 5, 6


DEBUG_DUMP = False


def build_kernel(tc, nch=NCH):
    nc = tc.nc
    dbg_outs = {}

    def dbg(name, ap, shape, dtype=F32):
        if not DEBUG_DUMP:
            return
        t = nc.dram_tensor(f"dbg_{name}", shape, dtype,
                           kind="ExternalOutput").ap()
        nc.gpsimd.dma_start(t, ap)
        dbg_outs[name] = t
    s_in = nc.dram_tensor("s", [nch * NB, A * S], F32, kind="ExternalInput").ap()
    wcat = nc.dram_tensor("wcat", [128, 2, NW], BF, kind="ExternalInput").ap()
    bcat = nc.dram_tensor("bcat", [128, 2, NBI], F32, kind="ExternalInput").ap()
    scal = nc.dram_tensor("scal", [1, 2], F32, kind="ExternalInput").ap()
    out = nc.dram_tensor("out", [nch * NB], F32, kind="ExternalOutput").ap()

    with contextlib.ExitStack() as ctx:
        const = ctx.enter_context(tc.tile_pool(name="const", bufs=1))
        stpool = ctx.enter_context(tc.tile_pool(name="stpool", bufs=8))
        snpool = ctx.enter_context(tc.tile_pool(name="snpool", bufs=4))
        apool = ctx.enter_context(tc.tile_pool(name="apool", bufs=8))
        tpool = ctx.enter_context(tc.tile_pool(name="tpool", bufs=2))
        mmout = ctx.enter_context(tc.tile_pool(name="mmout", bufs=2))
        kvpool = ctx.enter_context(tc.tile_pool(name="kvpool", bufs=2))
        fpool = ctx.enter_context(tc.tile_pool(name="fpool", bufs=1))
        zpool = ctx.enter_context(tc.tile_pool(name="zpool", bufs=2))
        onep = ctx.enter_context(tc.tile_pool(name="onep", bufs=1))
        trpool = ctx.enter_context(tc.tile_pool(name="trpool", bufs=2))
        psA = ctx.enter_context(tc.tile_pool(name="psA", bufs=2, space="PSUM"))
        psT = ctx.enter_context(tc.tile_pool(name="psT", bufs=2, space="PSUM"))
        psS = ctx.enter_context(tc.tile_pool(name="psS", bufs=1, space="PSUM"))

        wtile = const.tile([128, 2, NW], BF)
        nc.sync.dma_start(wtile[:], wcat)
        btile = const.tile([128, 2, NBI], F32)
        nc.sync.dma_start(btile[:], bcat)
        eps_t = const.tile([128, 1], F32)
        nc.vector.memset(eps_t[:], EPS)
        ident = const.tile([128, 128], BF)
        make_identity(nc, ident[:])
        sw_t = const.tile([128, 1], F32)   # sum(W3) broadcast over partitions
        nc.gpsimd.dma_start(sw_t[:], scal[0:1, 0:1].to_broadcast((128, 1)))
        b3_t = const.tile([128, 1], F32)   # b3 broadcast
        nc.gpsimd.dma_start(b3_t[:], scal[0:1, 1:2].to_broadcast((128, 1)))
        # per-chunk stat rows: [p, {W3.qr, sum qr, sum qr^2}, chunk, j]
        Fp = const.tile([128, 3, nch, 4], F32)

        def W(col, ks, mc=0, width=128):
            return wtile[:, ks, col + mc * 128: col + mc * 128 + width]

        def stage_a(c):
            # ---- Stage A: load (f32->bf16 cast in DMA) + LayerNorm ----
            sT, sN = [], []
            mv4 = apool.tile([128, 4, 2], F32, tag="mv")
            for bt in range(4):
                st = stpool.tile([128, A * S], BF, tag="s_in")
                nc.gpsimd.dma_start(
                    st[:], s_in[c * NB + bt * 128: c * NB + (bt + 1) * 128, :])
                sn = snpool.tile([128, A * S], BF, tag="sn")
                stats = apool.tile([128, 4, 6], F32, tag="stats")
                for g in range(4):
                    nc.vector.bn_stats(stats[:, g, :],
                                       st[:, g * 512:(g + 1) * 512])
                nc.vector.bn_aggr(mv4[:, bt], stats[:])
                sT.append(st)
                sN.append(sn)
            rt4 = apool.tile([128, 4], F32, tag="rt")
            w4 = apool.tile([128, 4], F32, tag="w4")
            nc.vector.tensor_scalar(w4[:], mv4[:, :, 1], scalar1=EPS,
                                    scalar2=-0.5, op0=OP.add, op1=OP.mult)
            nc.vector.tensor_scalar_add(rt4[:], w4[:], 1.5)
            t4 = apool.tile([128, 4], F32, tag="t4")
            nc.vector.tensor_mul(t4[:], rt4[:], rt4[:])
            nc.vector.tensor_mul(t4[:], t4[:], w4[:])
            nc.vector.tensor_scalar_add(t4[:], t4[:], 1.5)
            nc.vector.tensor_mul(rt4[:], rt4[:], t4[:])
            for bt in range(4):
                nc.vector.tensor_scalar(
                    sN[bt][:], sT[bt][:], scalar1=mv4[:, bt, 0:1],
                    scalar2=rt4[:, bt:bt + 1],
                    op0=OP.subtract, op1=OP.mult)
            return sT, sN

        pend = stage_a(0)
        for c in range(nch):
            sT, sN = pend

            # ---- Stage T: PE transpose to feature-major ----
            # snT[p, bt, fb, bb] = sn_bt[bb, fb*128+p]; feature f = fb*128+p,
            # fb = 2*a + ks (a=agent, ks=K-half); sample index = bt*128+bb.
            # bt-granular: each bt's 16 transposes start as soon as that
            # bt's LN apply lands (keeps the PE warm through stage A); a
            # psT tile packs 8 transposes into one full PSUM bank.
            snT = tpool.tile([128, 4, 16, 128], BF, tag="snT")
            for bt in range(4):
                for g in range(2):
                    pt = psT.tile([128, 8, 128], BF, tag="ptrans")
                    for j in range(8):
                        fb = g * 8 + j
                        nc.tensor.transpose(
                            pt[:, j, :], sN[bt][:, fb * 128:(fb + 1) * 128],
                            ident[:])
                    if bt == 0:
                        nc.scalar.activation(snT[:, bt, g * 8:(g + 1) * 8],
                                             pt[:], AF.Copy)
                    else:
                        nc.vector.tensor_copy(snT[:, bt, g * 8:(g + 1) * 8],
                                              pt[:])

            def rhs_s(ks, a):
                return snT[:, :, 2 * a + ks]   # [128, 4, 128] -> N=512

            # ---- projections (all contract over the 256 encoder inputs) ----
            def proj256(colbase, a, bias_slot, dst_mc_ap, func=AF.Identity):
                for mc in range(2):
                    ps = psA.tile([128, 2, NB], F32, tag="psmm")
                    for ks in range(2):
                        nc.tensor.matmul(
                            ps[:, 0], W(colbase, ks, mc), rhs_s(ks, a),
                            start=(ks == 0), stop=(ks == 1))
                    nc.scalar.activation(
                        dst_mc_ap(mc), ps[:, 0], func,
                        bias=btile[:, mc, bias_slot:bias_slot + 1])

            qT = mmout.tile([128, 2, NB], BF, tag="qT")
            proj256(C_WQ, 0, B_BQ, lambda mc: qT[:, mc])

            kT = kvpool.tile([128, 2, 7, NB], BF, tag="kT")
            vT = kvpool.tile([128, 2, 7, NB], BF, tag="vT")
            # a-pairs share one 2-bank PSUM tile and a single wide eviction
            apairs = [(1, 2), (3, 4), (5, 6), (7,)]
            for ap_ in apairs:
                for mc in range(2):
                    na = len(ap_)
                    psk = psA.tile([128, 2, NB], F32, tag="psmm")
                    for j, a in enumerate(ap_):
                        for ks in range(2):
                            nc.tensor.matmul(
                                psk[:, j], W(C_WK, ks, mc), rhs_s(ks, a),
                                start=(ks == 0), stop=(ks == 1))
                    nc.scalar.activation(
                        kT[:, mc, ap_[0] - 1:ap_[0] - 1 + na], psk[:, :na],
                        AF.Identity, bias=btile[:, mc, B_BK:B_BK + 1])
                    psv = psA.tile([128, 2, NB], F32, tag="psmm")
                    for j, a in enumerate(ap_):
                        for ks in range(2):
                            nc.tensor.matmul(
                                psv[:, j], W(C_WV, ks, mc), rhs_s(ks, a),
                                start=(ks == 0), stop=(ks == 1))
                    # leaky_relu(t, .01) = max(.01*t, t), t = x + bv
                    dst = vT[:, mc, ap_[0] - 1:ap_[0] - 1 + na]
                    nc.scalar.activation(
                        dst, psv[:, :na], AF.Identity,
                        bias=btile[:, mc, B_BV:B_BV + 1])
                    t2 = trpool.tile([128, 2, NB], BF, tag="vt2")
                    nc.vector.tensor_scalar_mul(t2[:, :na], dst, 0.01)
                    nc.vector.tensor_max(dst, dst, t2[:, :na])

            # ---- attention ----
            # qk = kT * qT in place (kT is dead after the score matmuls)
            qk = kT
            for k in range(7):
                nc.vector.tensor_mul(qk[:, :, k], kT[:, :, k], qT[:])
            # block-ones matmul: reduces QK over each head's 64 dims and
            # broadcasts the score back across them (diag chunks only).
            # exp writes back over qk (each qk[k] is dead after its score MM).
            eb = kT
            kpairs = [(0, 1), (2, 3), (4, 5), (6,)]
            for mc in range(2):
                for kp in kpairs:
                    nk = len(kp)
                    pss = psA.tile([128, 2, NB], F32, tag="psmm")
                    for j, k in enumerate(kp):
                        nc.tensor.matmul(
                            pss[:, j], W(C_L, mc, mc), qk[:, mc, k],
                            start=True, stop=True)
                    nc.scalar.activation(
                        eb[:, mc, kp[0]:kp[0] + nk], pss[:, :nk], AF.Exp,
                        scale=1.0 / np.sqrt(HD))

            # softmax denom, compact: Z[h] = sum_k exp(score_nk), per head.
            # ones/64 matvec reduces the broadcast eb over each head's 64
            # dims and accumulates over k in PSUM; one [2, NB] tile per mc
            # (PE output base must be 32-aligned, so two tiles at base 0).
            psZ = psS.tile([2, 2, NB], F32, tag="psZ")
            for mc in range(2):
                for k in range(7):
                    nc.tensor.matmul(
                        psZ[:, mc], W(C_Z, 0, 0, width=2),
                        eb[:, mc, k], start=(k == 0), stop=(k == 6))
            # both mc halves staged at partition base 0 (reciprocal_approx
            # is base-0 only): zc[:, mc] = Z of half mc
            zc = zpool.tile([2, 2, NB], F32, tag="zc")
            nc.scalar.activation(zc[:], psZ[:], AF.Identity)
            nc.vector.reciprocal_approx_fast(zc[:], zc[:])
            rcb = zpool.tile([2, 2, NB], BF, tag="rcb")
            nc.vector.tensor_copy(rcb[:], zc[:])
            # broadcast 1/Z back to all 256 dims: indicator matmul
            # (wb row d//64 -> dim d of half mc)
            psB = psA.tile([128, 2, NB], F32, tag="psmm")
            for mc in range(2):
                nc.tensor.matmul(
                    psB[:, mc],
                    wtile[0:2, 0, C_B + mc * 128:C_B + mc * 128 + 128],
                    rcb[:, mc], start=True, stop=True)

            # software pipeline: emit the next chunk's front-end here,
            # where the DVE otherwise idles waiting on evictions
            if c + 1 < nch:
                pend = stage_a(c + 1)

            # u = eb * vT in place (Z matvecs above already consumed eb)
            for k in range(7):
                nc.vector.tensor_mul(eb[:, :, k], eb[:, :, k], vT[:, :, k])

            # avU k-reduction: binary DVE add tree (short dependency chain;
            # an SDMA accumulate chain would serialize ~7x2us on the FIFO
            # gpsimd queue and block the next chunk's input loads)
            h1 = trpool.tile([128, 2, NB], BF, tag="h1")
            h2 = trpool.tile([128, 2, NB], BF, tag="h2")
            h3 = trpool.tile([128, 2, NB], BF, tag="h3")
            nc.vector.tensor_add(h1[:], eb[:, :, 0], eb[:, :, 1])
            nc.vector.tensor_add(h2[:], eb[:, :, 2], eb[:, :, 3])
            nc.vector.tensor_add(h3[:], eb[:, :, 4], eb[:, :, 5])
            nc.vector.tensor_add(h1[:], h1[:], h2[:])
            nc.vector.tensor_add(h3[:], h3[:], eb[:, :, 6])
            avU = onep.tile([128, 2, NB], BF, tag="avU")
            nc.vector.tensor_add(avU[:], h1[:], h3[:])

            avT = mmout.tile([128, 2, NB], BF, tag="avT")
            nc.vector.tensor_mul(avT[:], avU[:], psB[:])
            if c == 0:
                dbg("zc", zc[:], [2, 2, NB])
                dbg("rcb", rcb[:], [2, 2, NB], BF)
                dbg("avU", avU[:], [128, 2, NB], BF)
                dbg("avT", avT[:], [128, 2, NB], BF)
                dbg("vT", vT[:], [128, 2, 7, NB], BF)
                dbg("kT", kT[:], [128, 2, 7, NB], BF)
                dbg("qT", qT[:], [128, 2, NB], BF)

            # ---- l1 fused with fc_out: qr = relu(W1a.s_i + WoW1b.av + b1f) ----
            qr = mmout.tile([128, 2, NB], BF, tag="qr")
            for mc in range(2):
                ps = psA.tile([128, 2, NB], F32, tag="psmm")
                for ks in range(2):
                    nc.tensor.matmul(ps[:, 0], W(C_WEW1A, ks, mc), rhs_s(ks, 0),
                                     start=(ks == 0), stop=False)
                for ks in range(2):
                    nc.tensor.matmul(ps[:, 0], W(C_WO1B, ks, mc), avT[:, ks],
                                     start=False, stop=(ks == 1))
                nc.scalar.activation(qr[:, mc], ps[:, 0], AF.Relu,
                                     bias=btile[:, mc, B_B1F:B_B1F + 1])
            qr2 = onep.tile([128, 2, NB], BF, tag="qr2")
            nc.scalar.activation(qr2[:], qr[:], AF.Square)

            # ---- final LN+l3 stats via matvecs (reuse the psZ banks) ----
            psF = psS.tile([2, 2, NB], F32, tag="psZ")
            for ks in range(2):
                nc.tensor.matmul(
                    psF[:, 0], W(C_W3O, ks, 0, width=2), qr[:, ks],
                    start=(ks == 0), stop=(ks == 1))
            for ks in range(2):
                nc.tensor.matmul(
                    psF[0:1, 1], W(C_W3O + 1, ks, 0, width=1), qr2[:, ks],
                    start=(ks == 0), stop=(ks == 1))
            stmp1 = fpool.tile([2, 2, NB], F32, tag="stmp1")
            nc.scalar.activation(stmp1[:], psF[:], AF.Copy)
            # scatter row [1, 512] -> Fp[:, r, c, :] (sample = p*4 + j; the DMA
            # pairs the flat source stream with the partition-major dest)
            nc.sync.dma_start(Fp[:, 0, c, :], stmp1[0:1, 0, :])
            nc.sync.dma_start(Fp[:, 1, c, :], stmp1[1:2, 0, :])
            nc.sync.dma_start(Fp[:, 2, c, :], stmp1[0:1, 1, :])

        # ---- final LN+l3 math on [128, nch*4] ----
        FW = nch * 4
        w3qr = Fp[:, 0].rearrange("p c j -> p (c j)")
        sq = Fp[:, 1].rearrange("p c j -> p (c j)")
        sq2 = Fp[:, 2].rearrange("p c j -> p (c j)")
        m = fpool.tile([128, FW], F32, tag="fm")
        nc.scalar.mul(m[:], sq, 1.0 / H)
        ex2 = fpool.tile([128, FW], F32, tag="fe")
        nc.scalar.mul(ex2[:], sq2, 1.0 / H)
        var = fpool.tile([128, FW], F32, tag="fv")
        nc.vector.tensor_mul(var[:], m[:], m[:])
        nc.vector.tensor_sub(var[:], ex2[:], var[:])
        rstd = fpool.tile([128, FW], F32, tag="fr")
        nc.scalar.activation(rstd[:], var[:], AF.Sqrt, bias=eps_t[:])
        nc.vector.reciprocal(rstd[:], rstd[:])
        msw = fpool.tile([128, FW], F32, tag="fw")
        nc.vector.tensor_scalar_mul(msw[:], m[:], sw_t[:])
        res = fpool.tile([128, FW], F32, tag="fres")
        nc.vector.tensor_sub(res[:], w3qr, msw[:])
        nc.vector.tensor_mul(res[:], res[:], rstd[:])
        nc.vector.tensor_scalar_add(res[:], res[:], b3_t[:])
        nc.sync.dma_start(
            out.rearrange("(c p j) -> p c j", p=128, j=4),
            res.rearrange("p (c j) -> p c j", j=4))
    return nc


def _prepare_host(We, be, Wq, Wk, Wv, bv, Wo, bo, W1, b1, W3, b3):
    f = lambda x: np.asarray(x, dtype=np.float32)
    We, be, Wq, Wk, Wv, bv = f(We), f(be), f(Wq), f(Wk), f(Wv), f(bv)
    Wo, bo, W1, b1, W3, b3 = f(Wo), f(bo), f(W1), f(b1), f(W3), f(b3)
    WeQ, beQ = We @ Wq, be @ Wq
    WeK, beK = We @ Wk, be @ Wk
    WeV, beV = We @ Wv, be @ Wv + bv
    W1a, W1b = W1[:D], W1[D:]
    WeW1a = We @ W1a
    WoW1b, b1f = Wo @ W1b, b1 + bo @ W1b + be @ W1a
    L = np.zeros((H, H), np.float32)
    for n in range(NH):
        L[n * HD:(n + 1) * HD, n * HD:(n + 1) * HD] = 1.0
    w3o = np.zeros((H, 2), np.float32)
    w3o[:, 0] = W3[:, 0]
    w3o[:, 1] = 1.0
    # Wz: per-head ones/64 reducer [256, 2] (rows 0-127 only)
    wz = np.zeros((H, 2), np.float32)
    wz[0:64, 0] = 1.0 / HD
    wz[64:128, 1] = 1.0 / HD
    # Wb: head->dim broadcast indicator [256, 256]:
    # col (mc*128 + d): row = d//64 is 1 (same pattern both mc).
    wb = np.zeros((H, 2 * 128), np.float32)
    for mc in range(2):
        for dd in range(128):
            wb[dd // 64, mc * 128 + dd] = 1.0
    wfull = np.concatenate([WeW1a, WeQ, WeK, WeV, WoW1b, L, w3o, wz, wb],
                           axis=1)
    assert wfull.shape == (256, NW)
    wcat = np.ascontiguousarray(
        wfull.reshape(2, 128, NW).transpose(1, 0, 2)).astype(ml_dtypes.bfloat16)
    ones = np.ones(H, np.float32)
    bfull = np.stack([be, beQ, beK, beV, b1f, W3[:, 0], ones], axis=1)
    assert bfull.shape == (256, NBI)
    bcat = np.ascontiguousarray(bfull.reshape(2, 128, NBI).transpose(1, 0, 2))
    scal = np.array([[W3.sum(), b3[0]]], np.float32)
    return wcat, bcat, scal


_CACHED = {}


def _get_compiled(nch=NCH, num_devices=1):
    key = (nch, num_devices)
    if key not in _CACHED:
        nc = bacc.Bacc("TRN2", target_bir_lowering=False, debug=False,
                       num_devices=num_devices)
        with tile.TileContext(nc) as tc:
            build_kernel(tc, nch=nch)
        nc.compile()
        _CACHED[key] = nc
    return _CACHED[key]


def kernel(s, We, be, Wq, Wk, Wv, bv, Wo, bo, W1, b1, W3, b3, _trace=False):
    s = np.asarray(s, dtype=np.float32)
    wcat, bcat, scal = _prepare_host(We, be, Wq, Wk, Wv, bv, Wo, bo, W1, b1,
                                     W3, b3)
    nc = _get_compiled()
    in_maps = []
    for i in range(NCORES):
        shard = np.ascontiguousarray(s[i * BC:(i + 1) * BC])
        in_maps.append({"s": shard, "wcat": wcat, "bcat": bcat, "scal": scal})
    res = run_bass_kernel_spmd(nc, in_maps, core_ids=list(range(NCORES)),
                               trace=_trace)
    outs = [np.asarray(r["out"], np.float32).reshape(BC, 1)
            for r in res.results]
    full = np.concatenate(outs, axis=0)
    if _trace:
        return full, res
    return full


# revision 46
# speedup vs baseline: 1.1908x; 1.1908x over previous
"""Trainium2 Bass kernel for nn_Attention_Critic (gnn_message_passing).

Strategy (8-way batch data parallel, 4096 samples/core):
  - Host fuses weights: WeQ=We@Wq, WeK=We@Wk, WeV=We@Wv (encoder folded into
    the Q/K/V projections; enc is materialized only for agent 0),
    WoW1b=Wo@W1[256:], b1f=b1+bo@W1[256:] (fc_out folded into l1).
  - Final LN+l3 folded algebraically: res = rstd*(W3.qr - mean*sum(W3)) + b3,
    with mean/var of qr from ones/W3 matvecs (float32r).
  - Per 512-sample chunk: LN via bn_stats + tensor_scalar (sample-major),
    PE-transpose to feature-major [feat, sample] bf16, projections as
    128x128-tiled bf16 matmuls, attention scores via elementwise QK + a
    block-ones matmul that reduces over head dims AND broadcasts the score
    back across them, exp on ScalarE.
  - Softmax denominator computed compactly: per-head ones-matvecs reduce the
    broadcast exp over head dims AND accumulate over the 7 keys in PSUM;
    reciprocal_approx_fast on the compact [4, NB] form; PE indicator-matmul
    broadcasts 1/Z back to all 256 dims (no SDMA tree, no wide reciprocal).
  - avU k-reduction via SDMA CCE accumulate (frees DVE); leaky-relu as one
    scalar_tensor_tensor (mult+max) op.
"""

import contextlib

import numpy as np
import ml_dtypes

import concourse.bass as bass
import concourse.tile as tile
from concourse import bacc, mybir
from concourse.bass_utils import run_bass_kernel_spmd
from concourse.masks import make_identity

AF = mybir.ActivationFunctionType
OP = mybir.AluOpType
BF = mybir.dt.bfloat16
F32 = mybir.dt.float32

B, A, S, D, H, NH, HD = 32768, 8, 256, 256, 256, 4, 64
EPS = 1e-5
NCORES = 8
BC = B // NCORES          # 4096 samples per core
NB = 512                  # samples per chunk
NCH = BC // NB            # 8 chunks per core
NW = 1538 + 2 + 256       # fused bf16 weight columns (+Wz, +Wb)
NBI = 7                   # f32 bias/vec slots

# wcat column offsets (each 256 wide)
C_WEW1A, C_WQ, C_WK, C_WV, C_WO1B, C_L, C_W3O = (
    0, 256, 512, 768, 1024, 1280, 1536)
C_Z = 1538                # [128, 2] per-head ones/64 reducer (ks=0 plane)
C_B = 1540                # [64, 2*128] head->dim broadcast indicator
# bcat slots: biases 0-4, W3 at 5, ones at 6
B_BE, B_BQ, B_BK, B_BV, B_B1F, B_W3, B_ONE = 0, 1, 2, 3, 4, 5, 6


DEBUG_DUMP = False


def build_kernel(tc, nch=NCH):
    nc = tc.nc
    dbg_outs = {}

    def dbg(name, ap, shape, dtype=F32):
        if not DEBUG_DUMP:
            return
        t = nc.dram_tensor(f"dbg_{name}", shape, dtype,
                           kind="ExternalOutput").ap()
        nc.gpsimd.dma_start(t, ap)
        dbg_outs[name] = t
    s_in = nc.dram_tensor("s", [nch * NB, A * S], F32, kind="ExternalInput").ap()
    wcat = nc.dram_tensor("wcat", [128, 2, NW], BF, kind="ExternalInput").ap()
    bcat = nc.dram_tensor("bcat", [128, 2, NBI], F32, kind="ExternalInput").ap()
    scal = nc.dram_tensor("scal", [1, 2], F32, kind="ExternalInput").ap()
    out = nc.dram_tensor("out", [nch * NB], F32, kind="ExternalOutput").ap()

    with contextlib.ExitStack() as ctx:
        const = ctx.enter_context(tc.tile_pool(name="const", bufs=1))
        stpool = ctx.enter_context(tc.tile_pool(name="stpool", bufs=8))
        snpool = ctx.enter_context(tc.tile_pool(name="snpool", bufs=4))
        apool = ctx.enter_context(tc.tile_pool(name="apool", bufs=8))
        tpool = ctx.enter_context(tc.tile_pool(name="tpool", bufs=2))
        mmout = ctx.enter_context(tc.tile_pool(name="mmout", bufs=2))
        kvpool = ctx.enter_context(tc.tile_pool(name="kvpool", bufs=2))
        fpool = ctx.enter_context(tc.tile_pool(name="fpool", bufs=1))
        zpool = ctx.enter_context(tc.tile_pool(name="zpool", bufs=2))
        onep = ctx.enter_context(tc.tile_pool(name="onep", bufs=1))
        trpool = ctx.enter_context(tc.tile_pool(name="trpool", bufs=2))
        psA = ctx.enter_context(tc.tile_pool(name="psA", bufs=2, space="PSUM"))
        psT = ctx.enter_context(tc.tile_pool(name="psT", bufs=2, space="PSUM"))
        psS = ctx.enter_context(tc.tile_pool(name="psS", bufs=1, space="PSUM"))

        wtile = const.tile([128, 2, NW], BF)
        nc.sync.dma_start(wtile[:], wcat)
        btile = const.tile([128, 2, NBI], F32)
        nc.sync.dma_start(btile[:], bcat)
        eps_t = const.tile([128, 1], F32)
        nc.vector.memset(eps_t[:], EPS)
        ident = const.tile([128, 128], BF)
        make_identity(nc, ident[:])
        sw_t = const.tile([128, 1], F32)   # sum(W3) broadcast over partitions
        nc.gpsimd.dma_start(sw_t[:], scal[0:1, 0:1].to_broadcast((128, 1)))
        b3_t = const.tile([128, 1], F32)   # b3 broadcast
        nc.gpsimd.dma_start(b3_t[:], scal[0:1, 1:2].to_broadcast((128, 1)))
        # per-chunk stat rows: [p, {W3.qr, sum qr, sum qr^2}, chunk, j]
        Fp = const.tile([128, 3, nch, 4], F32)

        def W(col, ks, mc=0, width=128):
            return wtile[:, ks, col + mc * 128: col + mc * 128 + width]

        for c in range(nch):
            # ---- Stage A: load (f32->bf16 cast in DMA) + LayerNorm ----
            # fully per-bt (stats -> newton rsqrt -> apply) so each
            # 128-sample tile is ready for its transposes ~4us after its
            # load lands instead of waiting on the whole chunk's stats.
            sT, sN = [], []
            for bt in range(4):
                st = stpool.tile([128, A * S], BF, tag="s_in")
                nc.gpsimd.dma_start(
                    st[:], s_in[c * NB + bt * 128: c * NB + (bt + 1) * 128, :])
                sn = snpool.tile([128, A * S], BF, tag="sn")
                stats = apool.tile([128, 4, 6], F32, tag="stats")
                for g in range(4):
                    nc.vector.bn_stats(stats[:, g, :],
                                       st[:, g * 512:(g + 1) * 512])
                mv = apool.tile([128, 2], F32, tag="mv")
                nc.vector.bn_aggr(mv[:], stats[:])
                # rsqrt(v+eps) via 2 Newton steps from y0=1
                rt = apool.tile([128, 1], F32, tag="rt")
                w1 = apool.tile([128, 1], F32, tag="w1")
                nc.vector.tensor_scalar(w1[:], mv[:, 1:2], scalar1=EPS,
                                        scalar2=-0.5, op0=OP.add, op1=OP.mult)
                nc.vector.tensor_scalar_add(rt[:], w1[:], 1.5)
                t1 = apool.tile([128, 1], F32, tag="t1")
                nc.vector.tensor_mul(t1[:], rt[:], rt[:])
                nc.vector.tensor_mul(t1[:], t1[:], w1[:])
                nc.vector.tensor_scalar_add(t1[:], t1[:], 1.5)
                nc.vector.tensor_mul(rt[:], rt[:], t1[:])
                nc.vector.tensor_scalar(
                    sn[:], st[:], scalar1=mv[:, 0:1], scalar2=rt[:],
                    op0=OP.subtract, op1=OP.mult)
                sT.append(st)
                sN.append(sn)

            # ---- Stage T: PE transpose to feature-major ----
            # snT[p, bt, fb, bb] = sn_bt[bb, fb*128+p]; feature f = fb*128+p,
            # fb = 2*a + ks (a=agent, ks=K-half); sample index = bt*128+bb.
            # bt-granular: each bt's 16 transposes start as soon as that
            # bt's LN apply lands (keeps the PE warm through stage A); a
            # psT tile packs 8 transposes into one full PSUM bank.
            snT = tpool.tile([128, 4, 16, 128], BF, tag="snT")
            for bt in range(4):
                for g in range(2):
                    pt = psT.tile([128, 8, 128], BF, tag="ptrans")
                    for j in range(8):
                        fb = g * 8 + j
                        nc.tensor.transpose(
                            pt[:, j, :], sN[bt][:, fb * 128:(fb + 1) * 128],
                            ident[:])
                    if bt == 0:
                        nc.scalar.activation(snT[:, bt, g * 8:(g + 1) * 8],
                                             pt[:], AF.Copy)
                    else:
                        nc.vector.tensor_copy(snT[:, bt, g * 8:(g + 1) * 8],
                                              pt[:])

            def rhs_s(ks, a):
                return snT[:, :, 2 * a + ks]   # [128, 4, 128] -> N=512

            # ---- projections (all contract over the 256 encoder inputs) ----
            def proj256(colbase, a, bias_slot, dst_mc_ap, func=AF.Identity):
                for mc in range(2):
                    ps = psA.tile([128, 2, NB], F32, tag="psmm")
                    for ks in range(2):
                        nc.tensor.matmul(
                            ps[:, 0], W(colbase, ks, mc), rhs_s(ks, a),
                            start=(ks == 0), stop=(ks == 1))
                    nc.scalar.activation(
                        dst_mc_ap(mc), ps[:, 0], func,
                        bias=btile[:, mc, bias_slot:bias_slot + 1])

            qT = mmout.tile([128, 2, NB], BF, tag="qT")
            proj256(C_WQ, 0, B_BQ, lambda mc: qT[:, mc])

            kT = kvpool.tile([128, 2, 7, NB], BF, tag="kT")
            vT = kvpool.tile([128, 2, 7, NB], BF, tag="vT")
            # a-pairs share one 2-bank PSUM tile and a single wide eviction
            apairs = [(1, 2), (3, 4), (5, 6), (7,)]
            for ap_ in apairs:
                for mc in range(2):
                    na = len(ap_)
                    psk = psA.tile([128, 2, NB], F32, tag="psmm")
                    for j, a in enumerate(ap_):
                        for ks in range(2):
                            nc.tensor.matmul(
                                psk[:, j], W(C_WK, ks, mc), rhs_s(ks, a),
                                start=(ks == 0), stop=(ks == 1))
                    nc.scalar.activation(
                        kT[:, mc, ap_[0] - 1:ap_[0] - 1 + na], psk[:, :na],
                        AF.Identity, bias=btile[:, mc, B_BK:B_BK + 1])
                    psv = psA.tile([128, 2, NB], F32, tag="psmm")
                    for j, a in enumerate(ap_):
                        for ks in range(2):
                            nc.tensor.matmul(
                                psv[:, j], W(C_WV, ks, mc), rhs_s(ks, a),
                                start=(ks == 0), stop=(ks == 1))
                    # leaky_relu(t, .01) = max(.01*t, t), t = x + bv
                    dst = vT[:, mc, ap_[0] - 1:ap_[0] - 1 + na]
                    nc.scalar.activation(
                        dst, psv[:, :na], AF.Identity,
                        bias=btile[:, mc, B_BV:B_BV + 1])
                    t2 = trpool.tile([128, 2, NB], BF, tag="vt2")
                    nc.vector.tensor_scalar_mul(t2[:, :na], dst, 0.01)
                    nc.vector.tensor_max(dst, dst, t2[:, :na])

            # ---- attention ----
            # qk = kT * qT in place (kT is dead after the score matmuls)
            qk = kT
            for k in range(7):
                nc.vector.tensor_mul(qk[:, :, k], kT[:, :, k], qT[:])
            # block-ones matmul: reduces QK over each head's 64 dims and
            # broadcasts the score back across them (diag chunks only).
            # exp writes back over qk (each qk[k] is dead after its score MM).
            eb = kT
            kpairs = [(0, 1), (2, 3), (4, 5), (6,)]
            for mc in range(2):
                for kp in kpairs:
                    nk = len(kp)
                    pss = psA.tile([128, 2, NB], F32, tag="psmm")
                    for j, k in enumerate(kp):
                        nc.tensor.matmul(
                            pss[:, j], W(C_L, mc, mc), qk[:, mc, k],
                            start=True, stop=True)
                    nc.scalar.activation(
                        eb[:, mc, kp[0]:kp[0] + nk], pss[:, :nk], AF.Exp,
                        scale=1.0 / np.sqrt(HD))

            # softmax denom, compact: Z[h] = sum_k exp(score_nk), per head.
            # ones/64 matvec reduces the broadcast eb over each head's 64
            # dims and accumulates over k in PSUM; one [2, NB] tile per mc
            # (PE output base must be 32-aligned, so two tiles at base 0).
            psZ = psS.tile([2, 2, NB], F32, tag="psZ")
            for mc in range(2):
                for k in range(7):
                    nc.tensor.matmul(
                        psZ[:, mc], W(C_Z, 0, 0, width=2),
                        eb[:, mc, k], start=(k == 0), stop=(k == 6))
            # both mc halves staged at partition base 0 (reciprocal_approx
            # is base-0 only): zc[:, mc] = Z of half mc
            zc = zpool.tile([2, 2, NB], F32, tag="zc")
            nc.scalar.activation(zc[:], psZ[:], AF.Identity)
            nc.vector.reciprocal_approx_fast(zc[:], zc[:])
            rcb = zpool.tile([2, 2, NB], BF, tag="rcb")
            nc.vector.tensor_copy(rcb[:], zc[:])
            # broadcast 1/Z back to all 256 dims: indicator matmul
            # (wb row d//64 -> dim d of half mc)
            psB = psA.tile([128, 2, NB], F32, tag="psmm")
            for mc in range(2):
                nc.tensor.matmul(
                    psB[:, mc],
                    wtile[0:2, 0, C_B + mc * 128:C_B + mc * 128 + 128],
                    rcb[:, mc], start=True, stop=True)

            # u = eb * vT in place (Z matvecs above already consumed eb)
            for k in range(7):
                nc.vector.tensor_mul(eb[:, :, k], eb[:, :, k], vT[:, :, k])

            # avU k-reduction: binary DVE add tree (short dependency chain;
            # an SDMA accumulate chain would serialize ~7x2us on the FIFO
            # gpsimd queue and block the next chunk's input loads)
            h1 = trpool.tile([128, 2, NB], BF, tag="h1")
            h2 = trpool.tile([128, 2, NB], BF, tag="h2")
            h3 = trpool.tile([128, 2, NB], BF, tag="h3")
            nc.vector.tensor_add(h1[:], eb[:, :, 0], eb[:, :, 1])
            nc.vector.tensor_add(h2[:], eb[:, :, 2], eb[:, :, 3])
            nc.vector.tensor_add(h3[:], eb[:, :, 4], eb[:, :, 5])
            nc.vector.tensor_add(h1[:], h1[:], h2[:])
            nc.vector.tensor_add(h3[:], h3[:], eb[:, :, 6])
            avU = onep.tile([128, 2, NB], BF, tag="avU")
            nc.vector.tensor_add(avU[:], h1[:], h3[:])

            avT = mmout.tile([128, 2, NB], BF, tag="avT")
            nc.vector.tensor_mul(avT[:], avU[:], psB[:])
            if c == 0:
                dbg("zc", zc[:], [2, 2, NB])
                dbg("rcb", rcb[:], [2, 2, NB], BF)
                dbg("avU", avU[:], [128, 2, NB], BF)
                dbg("avT", avT[:], [128, 2, NB], BF)
                dbg("vT", vT[:], [128, 2, 7, NB], BF)
                dbg("kT", kT[:], [128, 2, 7, NB], BF)
                dbg("qT", qT[:], [128, 2, NB], BF)

            # ---- l1 fused with fc_out: qr = relu(W1a.s_i + WoW1b.av + b1f) ----
            qr = mmout.tile([128, 2, NB], BF, tag="qr")
            for mc in range(2):
                ps = psA.tile([128, 2, NB], F32, tag="psmm")
                for ks in range(2):
                    nc.tensor.matmul(ps[:, 0], W(C_WEW1A, ks, mc), rhs_s(ks, 0),
                                     start=(ks == 0), stop=False)
                for ks in range(2):
                    nc.tensor.matmul(ps[:, 0], W(C_WO1B, ks, mc), avT[:, ks],
                                     start=False, stop=(ks == 1))
                nc.scalar.activation(qr[:, mc], ps[:, 0], AF.Relu,
                                     bias=btile[:, mc, B_B1F:B_B1F + 1])
            qr2 = onep.tile([128, 2, NB], BF, tag="qr2")
            nc.scalar.activation(qr2[:], qr[:], AF.Square)

            # ---- final LN+l3 stats via matvecs (reuse the psZ banks) ----
            psF = psS.tile([2, 2, NB], F32, tag="psZ")
            for ks in range(2):
                nc.tensor.matmul(
                    psF[:, 0], W(C_W3O, ks, 0, width=2), qr[:, ks],
                    start=(ks == 0), stop=(ks == 1))
            for ks in range(2):
                nc.tensor.matmul(
                    psF[0:1, 1], W(C_W3O + 1, ks, 0, width=1), qr2[:, ks],
                    start=(ks == 0), stop=(ks == 1))
            stmp1 = fpool.tile([2, 2, NB], F32, tag="stmp1")
            nc.scalar.activation(stmp1[:], psF[:], AF.Copy)
            # scatter row [1, 512] -> Fp[:, r, c, :] (sample = p*4 + j; the DMA
            # pairs the flat source stream with the partition-major dest)
            nc.sync.dma_start(Fp[:, 0, c, :], stmp1[0:1, 0, :])
            nc.sync.dma_start(Fp[:, 1, c, :], stmp1[1:2, 0, :])
            nc.sync.dma_start(Fp[:, 2, c, :], stmp1[0:1, 1, :])

        # ---- final LN+l3 math on [128, nch*4] ----
        FW = nch * 4
        w3qr = Fp[:, 0].rearrange("p c j -> p (c j)")
        sq = Fp[:, 1].rearrange("p c j -> p (c j)")
        sq2 = Fp[:, 2].rearrange("p c j -> p (c j)")
        m = fpool.tile([128, FW], F32, tag="fm")
        nc.scalar.mul(m[:], sq, 1.0 / H)
        ex2 = fpool.tile([128, FW], F32, tag="fe")
        nc.scalar.mul(ex2[:], sq2, 1.0 / H)
        var = fpool.tile([128, FW], F32, tag="fv")
        nc.vector.tensor_mul(var[:], m[:], m[:])
        nc.vector.tensor_sub(var[:], ex2[:], var[:])
        rstd = fpool.tile([128, FW], F32, tag="fr")
        nc.scalar.activation(rstd[:], var[:], AF.Sqrt, bias=eps_t[:])
        nc.vector.reciprocal(rstd[:], rstd[:])
        msw = fpool.tile([128, FW], F32, tag="fw")
        nc.vector.tensor_scalar_mul(msw[:], m[:], sw_t[:])
        res = fpool.tile([128, FW], F32, tag="fres")
        nc.vector.tensor_sub(res[:], w3qr, msw[:])
        nc.vector.tensor_mul(res[:], res[:], rstd[:])
        nc.vector.tensor_scalar_add(res[:], res[:], b3_t[:])
        nc.sync.dma_start(
            out.rearrange("(c p j) -> p c j", p=128, j=4),
            res.rearrange("p (c j) -> p c j", j=4))
    return nc


def _prepare_host(We, be, Wq, Wk, Wv, bv, Wo, bo, W1, b1, W3, b3):
    f = lambda x: np.asarray(x, dtype=np.float32)
    We, be, Wq, Wk, Wv, bv = f(We), f(be), f(Wq), f(Wk), f(Wv), f(bv)
    Wo, bo, W1, b1, W3, b3 = f(Wo), f(bo), f(W1), f(b1), f(W3), f(b3)
    WeQ, beQ = We @ Wq, be @ Wq
    WeK, beK = We @ Wk, be @ Wk
    WeV, beV = We @ Wv, be @ Wv + bv
    W1a, W1b = W1[:D], W1[D:]
    WeW1a = We @ W1a
    WoW1b, b1f = Wo @ W1b, b1 + bo @ W1b + be @ W1a
    L = np.zeros((H, H), np.float32)
    for n in range(NH):
        L[n * HD:(n + 1) * HD, n * HD:(n + 1) * HD] = 1.0
    w3o = np.zeros((H, 2), np.float32)
    w3o[:, 0] = W3[:, 0]
    w3o[:, 1] = 1.0
    # Wz: per-head ones/64 reducer [256, 2] (rows 0-127 only)
    wz = np.zeros((H, 2), np.float32)
    wz[0:64, 0] = 1.0 / HD
    wz[64:128, 1] = 1.0 / HD
    # Wb: head->dim broadcast indicator [256, 256]:
    # col (mc*128 + d): row = d//64 is 1 (same pattern both mc).
    wb = np.zeros((H, 2 * 128), np.float32)
    for mc in range(2):
        for dd in range(128):
            wb[dd // 64, mc * 128 + dd] = 1.0
    wfull = np.concatenate([WeW1a, WeQ, WeK, WeV, WoW1b, L, w3o, wz, wb],
                           axis=1)
    assert wfull.shape == (256, NW)
    wcat = np.ascontiguousarray(
        wfull.reshape(2, 128, NW).transpose(1, 0, 2)).astype(ml_dtypes.bfloat16)
    ones = np.ones(H, np.float32)
    bfull = np.stack([be, beQ, beK, beV, b1f, W3[:, 0], ones], axis=1)
    assert bfull.shape == (256, NBI)
    bcat = np.ascontiguousarray(bfull.reshape(2, 128, NBI).transpose(1, 0, 2))
    scal = np.array([[W3.sum(), b3[0]]], np.float32)
    return wcat, bcat, scal


_CACHED = {}


def _get_compiled(nch=NCH, num_devices=1):
    key = (nch, num_devices)
    if key not in _CACHED:
        nc = bacc.Bacc("TRN2", target_bir_lowering=False, debug=False,
                       num_devices=num_devices)
        with tile.TileContext(nc) as tc:
            build_kernel(tc, nch=nch)
        nc.compile()
        _CACHED[key] = nc
    return _CACHED[key]


def kernel(s, We, be, Wq, Wk, Wv, bv, Wo, bo, W1, b1, W3, b3, _trace=False):
    s = np.asarray(s, dtype=np.float32)
    wcat, bcat, scal = _prepare_host(We, be, Wq, Wk, Wv, bv, Wo, bo, W1, b1,
                                     W3, b3)
    nc = _get_compiled()
    in_maps = []
    for i in range(NCORES):
        shard = np.ascontiguousarray(s[i * BC:(i + 1) * BC])
        in_maps.append({"s": shard, "wcat": wcat, "bcat": bcat, "scal": scal})
    res = run_bass_kernel_spmd(nc, in_maps, core_ids=list(range(NCORES)),
                               trace=_trace)
    outs = [np.asarray(r["out"], np.float32).reshape(BC, 1)
            for r in res.results]
    full = np.concatenate(outs, axis=0)
    if _trace:
        return full, res
    return full


# revision 47
# speedup vs baseline: 1.2408x; 1.0420x over previous
"""Trainium2 Bass kernel for nn_Attention_Critic (gnn_message_passing).

Strategy (8-way batch data parallel, 4096 samples/core):
  - Host fuses weights: WeQ=We@Wq, WeK=We@Wk, WeV=We@Wv (encoder folded into
    the Q/K/V projections; enc is materialized only for agent 0),
    WoW1b=Wo@W1[256:], b1f=b1+bo@W1[256:] (fc_out folded into l1).
  - Final LN+l3 folded algebraically: res = rstd*(W3.qr - mean*sum(W3)) + b3,
    with mean/var of qr from ones/W3 matvecs (float32r).
  - Per 512-sample chunk: LN via bn_stats + tensor_scalar (sample-major),
    PE-transpose to feature-major [feat, sample] bf16, projections as
    128x128-tiled bf16 matmuls, attention scores via elementwise QK + a
    block-ones matmul that reduces over head dims AND broadcasts the score
    back across them, exp on ScalarE.
  - Softmax denominator computed compactly: per-head ones-matvecs reduce the
    broadcast exp over head dims AND accumulate over the 7 keys in PSUM;
    reciprocal_approx_fast on the compact [4, NB] form; PE indicator-matmul
    broadcasts 1/Z back to all 256 dims (no SDMA tree, no wide reciprocal).
  - avU k-reduction via SDMA CCE accumulate (frees DVE); leaky-relu as one
    scalar_tensor_tensor (mult+max) op.
"""

import contextlib

import numpy as np
import ml_dtypes

import concourse.bass as bass
import concourse.tile as tile
from concourse import bacc, mybir
from concourse.bass_utils import run_bass_kernel_spmd
from concourse.masks import make_identity

AF = mybir.ActivationFunctionType
OP = mybir.AluOpType
BF = mybir.dt.bfloat16
F32 = mybir.dt.float32

B, A, S, D, H, NH, HD = 32768, 8, 256, 256, 256, 4, 64
EPS = 1e-5
NCORES = 8
BC = B // NCORES          # 4096 samples per core
NB = 512                  # samples per chunk
NCH = BC // NB            # 8 chunks per core
NW = 1538 + 2 + 256       # fused bf16 weight columns (+Wz, +Wb)
NBI = 7                   # f32 bias/vec slots

# wcat column offsets (each 256 wide)
C_WEW1A, C_WQ, C_WK, C_WV, C_WO1B, C_L, C_W3O = (
    0, 256, 512, 768, 1024, 1280, 1536)
C_Z = 1538                # [128, 2] per-head ones/64 reducer (ks=0 plane)
C_B = 1540                # [64, 2*128] head->dim broadcast indicator
# bcat slots: biases 0-4, W3 at 5, ones at 6
B_BE, B_BQ, B_BK, B_BV, B_B1F, B_W3, B_ONE = 0, 1, 2, 3, 4, 5, 6


DEBUG_DUMP = False


def build_kernel(tc, nch=NCH):
    nc = tc.nc
    dbg_outs = {}

    def dbg(name, ap, shape, dtype=F32):
        if not DEBUG_DUMP:
            return
        t = nc.dram_tensor(f"dbg_{name}", shape, dtype,
                           kind="ExternalOutput").ap()
        nc.gpsimd.dma_start(t, ap)
        dbg_outs[name] = t
    s_in = nc.dram_tensor("s", [nch * NB, A * S], F32, kind="ExternalInput").ap()
    wcat = nc.dram_tensor("wcat", [128, 2, NW], BF, kind="ExternalInput").ap()
    bcat = nc.dram_tensor("bcat", [128, 2, NBI], F32, kind="ExternalInput").ap()
    scal = nc.dram_tensor("scal", [1, 2], F32, kind="ExternalInput").ap()
    out = nc.dram_tensor("out", [nch * NB], F32, kind="ExternalOutput").ap()

    with contextlib.ExitStack() as ctx:
        const = ctx.enter_context(tc.tile_pool(name="const", bufs=1))
        stpool = ctx.enter_context(tc.tile_pool(name="stpool", bufs=8))
        snpool = ctx.enter_context(tc.tile_pool(name="snpool", bufs=4))
        apool = ctx.enter_context(tc.tile_pool(name="apool", bufs=8))
        tpool = ctx.enter_context(tc.tile_pool(name="tpool", bufs=2))
        mmout = ctx.enter_context(tc.tile_pool(name="mmout", bufs=2))
        kvpool = ctx.enter_context(tc.tile_pool(name="kvpool", bufs=2))
        fpool = ctx.enter_context(tc.tile_pool(name="fpool", bufs=1))
        zpool = ctx.enter_context(tc.tile_pool(name="zpool", bufs=2))
        onep = ctx.enter_context(tc.tile_pool(name="onep", bufs=1))
        trpool = ctx.enter_context(tc.tile_pool(name="trpool", bufs=2))
        psA = ctx.enter_context(tc.tile_pool(name="psA", bufs=2, space="PSUM"))
        psT = ctx.enter_context(tc.tile_pool(name="psT", bufs=2, space="PSUM"))
        psS = ctx.enter_context(tc.tile_pool(name="psS", bufs=1, space="PSUM"))

        wtile = const.tile([128, 2, NW], BF)
        nc.sync.dma_start(wtile[:], wcat)
        btile = const.tile([128, 2, NBI], F32)
        nc.sync.dma_start(btile[:], bcat)
        eps_t = const.tile([128, 1], F32)
        nc.vector.memset(eps_t[:], EPS)
        ident = const.tile([128, 128], BF)
        make_identity(nc, ident[:])
        sw_t = const.tile([128, 1], F32)   # sum(W3) broadcast over partitions
        nc.gpsimd.dma_start(sw_t[:], scal[0:1, 0:1].to_broadcast((128, 1)))
        b3_t = const.tile([128, 1], F32)   # b3 broadcast
        nc.gpsimd.dma_start(b3_t[:], scal[0:1, 1:2].to_broadcast((128, 1)))
        # per-chunk stat rows: [p, {W3.qr, sum qr, sum qr^2}, chunk, j]
        Fp = const.tile([128, 3, nch, 4], F32)

        def W(col, ks, mc=0, width=128):
            return wtile[:, ks, col + mc * 128: col + mc * 128 + width]

        for c in range(nch):
            # ---- Stage A: load (f32->bf16 cast in DMA) + LayerNorm ----
            # fully per-bt (stats -> newton rsqrt -> apply) so each
            # 128-sample tile is ready for its transposes ~4us after its
            # load lands instead of waiting on the whole chunk's stats.
            sT, sN = [], []
            mv4 = apool.tile([128, 4, 2], F32, tag="mv")
            for bt in range(4):
                st = stpool.tile([128, A * S], BF, tag="s_in")
                nc.gpsimd.dma_start(
                    st[:], s_in[c * NB + bt * 128: c * NB + (bt + 1) * 128, :])
                sn = snpool.tile([128, A * S], BF, tag="sn")
                stats = apool.tile([128, 4, 6], F32, tag="stats")
                for g in range(4):
                    nc.vector.bn_stats(stats[:, g, :],
                                       st[:, g * 512:(g + 1) * 512])
                nc.vector.bn_aggr(mv4[:, bt], stats[:])
                sT.append(st)
                sN.append(sn)
            # rsqrt(v+eps) via 2 Newton steps from y0=1 (batched over bts)
            rt4 = apool.tile([128, 4], F32, tag="rt")
            w4 = apool.tile([128, 4], F32, tag="w4")
            nc.vector.tensor_scalar(w4[:], mv4[:, :, 1], scalar1=EPS,
                                    scalar2=-0.5, op0=OP.add, op1=OP.mult)
            nc.vector.tensor_scalar_add(rt4[:], w4[:], 1.5)
            t4 = apool.tile([128, 4], F32, tag="t4")
            nc.vector.tensor_mul(t4[:], rt4[:], rt4[:])
            nc.vector.tensor_mul(t4[:], t4[:], w4[:])
            nc.vector.tensor_scalar_add(t4[:], t4[:], 1.5)
            nc.vector.tensor_mul(rt4[:], rt4[:], t4[:])
            for bt in range(4):
                nc.vector.tensor_scalar(
                    sN[bt][:], sT[bt][:], scalar1=mv4[:, bt, 0:1],
                    scalar2=rt4[:, bt:bt + 1],
                    op0=OP.subtract, op1=OP.mult)

            # ---- Stage T: PE transpose to feature-major ----
            # snT[p, bt, fb, bb] = sn_bt[bb, fb*128+p]; feature f = fb*128+p,
            # fb = 2*a + ks (a=agent, ks=K-half); sample index = bt*128+bb.
            # bt-granular: each bt's 16 transposes start as soon as that
            # bt's LN apply lands (keeps the PE warm through stage A); a
            # psT tile packs 8 transposes into one full PSUM bank.
            snT = tpool.tile([128, 4, 16, 128], BF, tag="snT")
            for bt in range(4):
                for g in range(2):
                    pt = psT.tile([128, 8, 128], BF, tag="ptrans")
                    for j in range(8):
                        fb = g * 8 + j
                        nc.tensor.transpose(
                            pt[:, j, :], sN[bt][:, fb * 128:(fb + 1) * 128],
                            ident[:])
                    if bt == 0:
                        nc.scalar.activation(snT[:, bt, g * 8:(g + 1) * 8],
                                             pt[:], AF.Copy)
                    else:
                        nc.vector.tensor_copy(snT[:, bt, g * 8:(g + 1) * 8],
                                              pt[:])

            def rhs_s(ks, a):
                return snT[:, :, 2 * a + ks]   # [128, 4, 128] -> N=512

            # ---- projections (all contract over the 256 encoder inputs) ----
            def proj256(colbase, a, bias_slot, dst_mc_ap, func=AF.Identity):
                for mc in range(2):
                    ps = psA.tile([128, 2, NB], F32, tag="psmm")
                    for ks in range(2):
                        nc.tensor.matmul(
                            ps[:, 0], W(colbase, ks, mc), rhs_s(ks, a),
                            start=(ks == 0), stop=(ks == 1))
                    nc.scalar.activation(
                        dst_mc_ap(mc), ps[:, 0], func,
                        bias=btile[:, mc, bias_slot:bias_slot + 1])

            qT = mmout.tile([128, 2, NB], BF, tag="qT")
            proj256(C_WQ, 0, B_BQ, lambda mc: qT[:, mc])

            kT = kvpool.tile([128, 2, 7, NB], BF, tag="kT")
            vT = kvpool.tile([128, 2, 7, NB], BF, tag="vT")
            # a-pairs share one 2-bank PSUM tile and a single wide eviction
            apairs = [(1, 2), (3, 4), (5, 6), (7,)]
            for ap_ in apairs:
                for mc in range(2):
                    na = len(ap_)
                    psk = psA.tile([128, 2, NB], F32, tag="psmm")
                    for j, a in enumerate(ap_):
                        for ks in range(2):
                            nc.tensor.matmul(
                                psk[:, j], W(C_WK, ks, mc), rhs_s(ks, a),
                                start=(ks == 0), stop=(ks == 1))
                    nc.scalar.activation(
                        kT[:, mc, ap_[0] - 1:ap_[0] - 1 + na], psk[:, :na],
                        AF.Identity, bias=btile[:, mc, B_BK:B_BK + 1])
                    psv = psA.tile([128, 2, NB], F32, tag="psmm")
                    for j, a in enumerate(ap_):
                        for ks in range(2):
                            nc.tensor.matmul(
                                psv[:, j], W(C_WV, ks, mc), rhs_s(ks, a),
                                start=(ks == 0), stop=(ks == 1))
                    # leaky_relu(t, .01) = max(.01*t, t), t = x + bv
                    dst = vT[:, mc, ap_[0] - 1:ap_[0] - 1 + na]
                    nc.scalar.activation(
                        dst, psv[:, :na], AF.Identity,
                        bias=btile[:, mc, B_BV:B_BV + 1])
                    t2 = trpool.tile([128, 2, NB], BF, tag="vt2")
                    nc.vector.tensor_scalar_mul(t2[:, :na], dst, 0.01)
                    nc.vector.tensor_max(dst, dst, t2[:, :na])

            # ---- attention ----
            # qk = kT * qT in place (kT is dead after the score matmuls)
            qk = kT
            for k in range(7):
                nc.vector.tensor_mul(qk[:, :, k], kT[:, :, k], qT[:])
            # block-ones matmul: reduces QK over each head's 64 dims and
            # broadcasts the score back across them (diag chunks only).
            # exp writes back over qk (each qk[k] is dead after its score MM).
            eb = kT
            kpairs = [(0, 1), (2, 3), (4, 5), (6,)]
            for mc in range(2):
                for kp in kpairs:
                    nk = len(kp)
                    pss = psA.tile([128, 2, NB], F32, tag="psmm")
                    for j, k in enumerate(kp):
                        nc.tensor.matmul(
                            pss[:, j], W(C_L, mc, mc), qk[:, mc, k],
                            start=True, stop=True)
                    nc.scalar.activation(
                        eb[:, mc, kp[0]:kp[0] + nk], pss[:, :nk], AF.Exp,
                        scale=1.0 / np.sqrt(HD))

            # softmax denom, compact: Z[h] = sum_k exp(score_nk), per head.
            # ones/64 matvec reduces the broadcast eb over each head's 64
            # dims and accumulates over k in PSUM; one [2, NB] tile per mc
            # (PE output base must be 32-aligned, so two tiles at base 0).
            psZ = psS.tile([2, 2, NB], F32, tag="psZ")
            for mc in range(2):
                for k in range(7):
                    nc.tensor.matmul(
                        psZ[:, mc], W(C_Z, 0, 0, width=2),
                        eb[:, mc, k], start=(k == 0), stop=(k == 6))
            # reciprocal straight off PSUM (base-0 only), cast to bf16
            zc = zpool.tile([2, 2, NB], F32, tag="zc")
            nc.vector.reciprocal_approx_fast(zc[:], psZ[:])
            rcb = zpool.tile([2, 2, NB], BF, tag="rcb")
            nc.vector.tensor_copy(rcb[:], zc[:])
            # broadcast 1/Z back to all 256 dims: indicator matmul
            # (wb row d//64 -> dim d of half mc)
            psB = psA.tile([128, 2, NB], F32, tag="psmm")
            for mc in range(2):
                nc.tensor.matmul(
                    psB[:, mc],
                    wtile[0:2, 0, C_B + mc * 128:C_B + mc * 128 + 128],
                    rcb[:, mc], start=True, stop=True)

            # u = eb * vT in place (Z matvecs above already consumed eb)
            for k in range(7):
                nc.vector.tensor_mul(eb[:, :, k], eb[:, :, k], vT[:, :, k])

            # avU k-reduction: binary DVE add tree (short dependency chain;
            # an SDMA accumulate chain would serialize ~7x2us on the FIFO
            # gpsimd queue and block the next chunk's input loads)
            h1 = trpool.tile([128, 2, NB], BF, tag="h1")
            h2 = trpool.tile([128, 2, NB], BF, tag="h2")
            h3 = trpool.tile([128, 2, NB], BF, tag="h3")
            nc.vector.tensor_add(h1[:], eb[:, :, 0], eb[:, :, 1])
            nc.vector.tensor_add(h2[:], eb[:, :, 2], eb[:, :, 3])
            nc.vector.tensor_add(h3[:], eb[:, :, 4], eb[:, :, 5])
            nc.vector.tensor_add(h1[:], h1[:], h2[:])
            nc.vector.tensor_add(h3[:], h3[:], eb[:, :, 6])
            avU = onep.tile([128, 2, NB], BF, tag="avU")
            nc.vector.tensor_add(avU[:], h1[:], h3[:])

            rsw = mmout.tile([128, 2, NB], BF, tag="rsw")
            nc.scalar.activation(rsw[:], psB[:], AF.Identity)
            avT = mmout.tile([128, 2, NB], BF, tag="avT")
            nc.vector.tensor_mul(avT[:], avU[:], rsw[:])
            if c == 0:
                dbg("zc", zc[:], [2, 2, NB])
                dbg("rcb", rcb[:], [2, 2, NB], BF)
                dbg("avU", avU[:], [128, 2, NB], BF)
                dbg("avT", avT[:], [128, 2, NB], BF)
                dbg("vT", vT[:], [128, 2, 7, NB], BF)
                dbg("kT", kT[:], [128, 2, 7, NB], BF)
                dbg("qT", qT[:], [128, 2, NB], BF)

            # ---- l1 fused with fc_out: qr = relu(W1a.s_i + WoW1b.av + b1f) ----
            qr = mmout.tile([128, 2, NB], BF, tag="qr")
            for mc in range(2):
                ps = psA.tile([128, 2, NB], F32, tag="psmm")
                for ks in range(2):
                    nc.tensor.matmul(ps[:, 0], W(C_WEW1A, ks, mc), rhs_s(ks, 0),
                                     start=(ks == 0), stop=False)
                for ks in range(2):
                    nc.tensor.matmul(ps[:, 0], W(C_WO1B, ks, mc), avT[:, ks],
                                     start=False, stop=(ks == 1))
                nc.scalar.activation(qr[:, mc], ps[:, 0], AF.Relu,
                                     bias=btile[:, mc, B_B1F:B_B1F + 1])
            qr2 = onep.tile([128, 2, NB], BF, tag="qr2")
            nc.scalar.activation(qr2[:], qr[:], AF.Square)

            # ---- final LN+l3 stats via matvecs (reuse the psZ banks) ----
            psF = psS.tile([2, 2, NB], F32, tag="psZ")
            for ks in range(2):
                nc.tensor.matmul(
                    psF[:, 0], W(C_W3O, ks, 0, width=2), qr[:, ks],
                    start=(ks == 0), stop=(ks == 1))
            for ks in range(2):
                nc.tensor.matmul(
                    psF[0:1, 1], W(C_W3O + 1, ks, 0, width=1), qr2[:, ks],
                    start=(ks == 0), stop=(ks == 1))
            stmp1 = fpool.tile([2, 2, NB], F32, tag="stmp1")
            nc.scalar.activation(stmp1[:], psF[:], AF.Copy)
            # scatter row [1, 512] -> Fp[:, r, c, :] (sample = p*4 + j; the DMA
            # pairs the flat source stream with the partition-major dest)
            nc.sync.dma_start(Fp[:, 0, c, :], stmp1[0:1, 0, :])
            nc.sync.dma_start(Fp[:, 1, c, :], stmp1[1:2, 0, :])
            nc.sync.dma_start(Fp[:, 2, c, :], stmp1[0:1, 1, :])

        # ---- final LN+l3 math on [128, nch*4] ----
        FW = nch * 4
        w3qr = Fp[:, 0].rearrange("p c j -> p (c j)")
        sq = Fp[:, 1].rearrange("p c j -> p (c j)")
        sq2 = Fp[:, 2].rearrange("p c j -> p (c j)")
        m = fpool.tile([128, FW], F32, tag="fm")
        nc.scalar.mul(m[:], sq, 1.0 / H)
        ex2 = fpool.tile([128, FW], F32, tag="fe")
        nc.scalar.mul(ex2[:], sq2, 1.0 / H)
        var = fpool.tile([128, FW], F32, tag="fv")
        nc.vector.tensor_mul(var[:], m[:], m[:])
        nc.vector.tensor_sub(var[:], ex2[:], var[:])
        rstd = fpool.tile([128, FW], F32, tag="fr")
        nc.scalar.activation(rstd[:], var[:], AF.Sqrt, bias=eps_t[:])
        nc.vector.reciprocal(rstd[:], rstd[:])
        msw = fpool.tile([128, FW], F32, tag="fw")
        nc.vector.tensor_scalar_mul(msw[:], m[:], sw_t[:])
        res = fpool.tile([128, FW], F32, tag="fres")
        nc.vector.tensor_sub(res[:], w3qr, msw[:])
        nc.vector.tensor_mul(res[:], res[:], rstd[:])
        nc.vector.tensor_scalar_add(res[:], res[:], b3_t[:])
        nc.sync.dma_start(
            out.rearrange("(c p j) -> p c j", p=128, j=4),
            res.rearrange("p (c j) -> p c j", j=4))
    return nc


def _prepare_host(We, be, Wq, Wk, Wv, bv, Wo, bo, W1, b1, W3, b3):
    f = lambda x: np.asarray(x, dtype=np.float32)
    We, be, Wq, Wk, Wv, bv = f(We), f(be), f(Wq), f(Wk), f(Wv), f(bv)
    Wo, bo, W1, b1, W3, b3 = f(Wo), f(bo), f(W1), f(b1), f(W3), f(b3)
    WeQ, beQ = We @ Wq, be @ Wq
    WeK, beK = We @ Wk, be @ Wk
    WeV, beV = We @ Wv, be @ Wv + bv
    W1a, W1b = W1[:D], W1[D:]
    WeW1a = We @ W1a
    WoW1b, b1f = Wo @ W1b, b1 + bo @ W1b + be @ W1a
    L = np.zeros((H, H), np.float32)
    for n in range(NH):
        L[n * HD:(n + 1) * HD, n * HD:(n + 1) * HD] = 1.0
    w3o = np.zeros((H, 2), np.float32)
    w3o[:, 0] = W3[:, 0]
    w3o[:, 1] = 1.0
    # Wz: per-head ones/64 reducer [256, 2] (rows 0-127 only)
    wz = np.zeros((H, 2), np.float32)
    wz[0:64, 0] = 1.0 / HD
    wz[64:128, 1] = 1.0 / HD
    # Wb: head->dim broadcast indicator [256, 256]:
    # col (mc*128 + d): row = d//64 is 1 (same pattern both mc).
    wb = np.zeros((H, 2 * 128), np.float32)
    for mc in range(2):
        for dd in range(128):
            wb[dd // 64, mc * 128 + dd] = 1.0
    wfull = np.concatenate([WeW1a, WeQ, WeK, WeV, WoW1b, L, w3o, wz, wb],
                           axis=1)
    assert wfull.shape == (256, NW)
    wcat = np.ascontiguousarray(
        wfull.reshape(2, 128, NW).transpose(1, 0, 2)).astype(ml_dtypes.bfloat16)
    ones = np.ones(H, np.float32)
    bfull = np.stack([be, beQ, beK, beV, b1f, W3[:, 0], ones], axis=1)
    assert bfull.shape == (256, NBI)
    bcat = np.ascontiguousarray(bfull.reshape(2, 128, NBI).transpose(1, 0, 2))
    scal = np.array([[W3.sum(), b3[0]]], np.float32)
    return wcat, bcat, scal


_CACHED = {}


def _get_compiled(nch=NCH, num_devices=1):
    key = (nch, num_devices)
    if key not in _CACHED:
        nc = bacc.Bacc("TRN2", target_bir_lowering=False, debug=False,
                       num_devices=num_devices)
        with tile.TileContext(nc) as tc:
            build_kernel(tc, nch=nch)
        nc.compile()
        _CACHED[key] = nc
    return _CACHED[key]


def kernel(s, We, be, Wq, Wk, Wv, bv, Wo, bo, W1, b1, W3, b3, _trace=False):
    s = np.asarray(s, dtype=np.float32)
    wcat, bcat, scal = _prepare_host(We, be, Wq, Wk, Wv, bv, Wo, bo, W1, b1,
                                     W3, b3)
    nc = _get_compiled()
    in_maps = []
    for i in range(NCORES):
        shard = np.ascontiguousarray(s[i * BC:(i + 1) * BC])
        in_maps.append({"s": shard, "wcat": wcat, "bcat": bcat, "scal": scal})
    res = run_bass_kernel_spmd(nc, in_maps, core_ids=list(range(NCORES)),
                               trace=_trace)
    outs = [np.asarray(r["out"], np.float32).reshape(BC, 1)
            for r in res.results]
    full = np.concatenate(outs, axis=0)
    if _trace:
        return full, res
    return full
